# revision 5
# baseline (speedup 1.0000x reference)
"""Trainium2 Bass kernel for single-head attention (B=8, S=2048, DIN=768, DOUT=64).

Strategy: data parallel — one batch element per NeuronCore (8 cores).
Per core, attention runs in transposed-score layout (k on partitions, q on
free dim), Scalar-engine-paced at ~1.11us per [128,1024] exp:

  qk proj   fp8e4 DoubleRow matmuls (x and Wqk prepacked on host into
            [p, 2, f] DR layout over din pairs) -> PSUM [q|k, s-cols]
  repack    DVE copies PSUM quarters into q_dr/k_dr [32, 2, S] fp8e4
            (DR layout over d pairs, bias added per-partition)
  mask      additive -2048 bias, fp8e5, applied by a DoubleRow PE matmul
            (identity stationary) accumulating into the score PSUM --
            exp(scale*(s-2048)) == 0, so no vector-engine mask multiply
  scores    fp8e4 DoubleRow matmul (2x rate), accumulated on the mask bias
  exp       ScalarE activation, PSUM fp32 -> SBUF bf16 (the bottleneck:
            32 x [128,1024] = ~35.6us busy)
  ctx       bf16 matmul, v65 = [v | 1] stationary (row 64 = softmax denom)
  epilogue  PE transposes + reciprocal + scale, interleaved into pass 1

Loop is q-half-major (2 passes x 16 k-tiles) so ctx PSUM is 2 banks and the
score PSUM can triple-buffer (3x2 banks): PSUM = 3*2 + 2 = 8 banks, which
keeps ScalarE gap-free. v projections (bf16, x held separately in bf16 --
fp8 v would cost ~2.4% output error) are interleaved into pass 0 so their
LDWEIGHTS hide under main-loop matmuls.
"""

import math
import sys
from contextlib import ExitStack

import numpy as np

sys.path.insert(0, "/opt/trn_rl_repo")

import ml_dtypes  # noqa: E402

import concourse.bass as bass  # noqa: E402
import concourse.tile as tile  # noqa: E402
from concourse import bacc, mybir  # noqa: E402
from concourse.bass import ds  # noqa: E402
from concourse.bass_utils import run_bass_kernel_spmd  # noqa: E402
from concourse.masks import make_identity  # noqa: E402

B, S, DIN, DOUT = 8, 2048, 768, 64
P = 128
NJ = 3  # din chunk-pairs for the DR qk projection
NCH = 6  # din chunks for the bf16 v projection
KT = S // P  # 16 key tiles
NB = 4  # qk projection column blocks of 512
NS = 512  # matmul moving free dim (one PSUM bank fp32)
H = 2  # q halves (passes)
HQ = S // H  # 1024

F32 = mybir.dt.float32
BF16 = mybir.dt.bfloat16
FP8E4 = mybir.dt.float8e4
FP8E5 = mybir.dt.float8e5
DR = mybir.MatmulPerfMode.DoubleRow

_NC_CACHE = None


def build_nc():
    nc = bacc.Bacc("TRN2", target_bir_lowering=False, debug=False)

    xdr = nc.declare_dram_parameter("xdr", [NJ, NB, P, 2, NS], FP8E4, isOutput=False)
    xbf = nc.declare_dram_parameter("xbf", [NCH, P, S], BF16, isOutput=False)
    mb = nc.declare_dram_parameter("mb", [KT, 64, 2, S], FP8E5, isOutput=False)
    wqk = nc.declare_dram_parameter("wqk", [NJ, P, 2, P], FP8E4, isOutput=False)
    wv = nc.declare_dram_parameter("wv", [NCH, P, DOUT], BF16, isOutput=False)
    idr = nc.declare_dram_parameter("idr", [64, 2, P], FP8E5, isOutput=False)
    bqk = nc.declare_dram_parameter("bqk", [P, 1], F32, isOutput=False)
    out = nc.declare_dram_parameter("out", [S, DOUT], F32, isOutput=True)

    inv_sqrt_s = float(1.0 / math.sqrt(S))

    with tile.TileContext(nc) as tc, ExitStack() as ctx:
        singles = ctx.enter_context(tc.tile_pool(name="singles", bufs=1))
        epool = ctx.enter_context(tc.tile_pool(name="epool", bufs=3))
        opool = ctx.enter_context(tc.tile_pool(name="opool", bufs=4))

        # ---- constants / weights (small DMAs first)
        wqk_sb = singles.tile([P, NJ, 2, P], FP8E4)
        nc.sync.dma_start(out=wqk_sb, in_=wqk.rearrange("j p g m -> p j g m"))
        idr_sb = singles.tile([64, 2, P], FP8E5)
        nc.sync.dma_start(out=idr_sb, in_=idr[:, :, :])
        bqk_sb = singles.tile([P, 1], F32)
        nc.sync.dma_start(out=bqk_sb, in_=bqk[:, :])
        wv_sb = singles.tile([P, NCH, DOUT], BF16)
        nc.sync.dma_start(out=wv_sb, in_=wv.rearrange("c p m -> p c m"))

        # ---- x in DR fp8 layout for the qk projection
        xdr_sb = singles.tile([P, NJ, NB, 2, NS], FP8E4)
        for blk in range(NB):
            for j in range(NJ):
                nc.sync.dma_start(
                    out=xdr_sb[:, j, blk, :, :], in_=xdr[j, blk, :, :, :]
                )

        # ---- x in bf16 for the v projection
        xbf_sb = singles.tile([P, NCH, S], BF16)
        for c in range(NCH):
            nc.sync.dma_start(out=xbf_sb[:, c, :], in_=xbf[c, :, :])

        # ---- mask bias (-2048 where masked), DR fp8e5 layout, per key tile
        mb_sb = singles.tile([64, KT, 2, S], FP8E5)
        for t in range(KT):
            nc.sync.dma_start(out=mb_sb[:, t, :, :], in_=mb[t, :, :, :])

        ident = singles.tile([P, P], F32)
        make_identity(nc, ident)

        # ---- v with a ones column: [s(128 part), ktile, 65] bf16
        v65_sb = singles.tile([P, KT, DOUT + 1], BF16)
        nc.gpsimd.memset(v65_sb, 1.0)

        q_dr = singles.tile([32, 2, S], FP8E4)
        k_dr = singles.tile([32, 2, S], FP8E4)
        ctxT_sb = singles.tile([DOUT + 1, S], F32)

        with (
            tc.tile_pool(name="psS", bufs=3, space="PSUM") as psS,
            tc.tile_pool(name="psC", bufs=1, space="PSUM") as psC,
        ):

            def vproj(t):
                v_ps = psS.tile([P, HQ], F32, tag="big")
                for c in range(NCH):
                    nc.tensor.matmul(
                        v_ps[:, 0:DOUT],
                        lhsT=xbf_sb[:, c, ds(t * P, P)],
                        rhs=wv_sb[:, c, :],
                        start=(c == 0),
                        stop=(c == NCH - 1),
                    )
                nc.vector.tensor_copy(v65_sb[:, t, 0:DOUT], v_ps[:, 0:DOUT])

            # ---- qk projection (fp8 DR) + repack to q_dr/k_dr
            for blk in range(NB):
                qk_ps = psS.tile([P, HQ], F32, tag="big")
                for j in range(NJ):
                    nc.tensor.matmul(
                        qk_ps[:, 0:NS],
                        lhsT=wqk_sb[:, j, :, :],
                        rhs=xdr_sb[:, j, blk, :, :],
                        start=(j == 0),
                        stop=(j == NJ - 1),
                        perf_mode=DR,
                    )
                cols = ds(blk * NS, NS)
                nc.vector.tensor_scalar_add(
                    q_dr[:, 0, cols], qk_ps[0:32, 0:NS], bqk_sb[0:32]
                )
                nc.vector.tensor_scalar_add(
                    q_dr[:, 1, cols], qk_ps[32:64, 0:NS], bqk_sb[32:64]
                )
                nc.vector.tensor_scalar_add(
                    k_dr[:, 0, cols], qk_ps[64:96, 0:NS], bqk_sb[64:96]
                )
                nc.vector.tensor_scalar_add(
                    k_dr[:, 1, cols], qk_ps[96:128, 0:NS], bqk_sb[96:128]
                )

            vproj(0)

            # ---- epilogue worker: transpose back, normalize, stage stores
            ostage = {}

            def epilogue_tile(h, qt):
                tr = psS.tile([P, HQ], F32, tag="big")
                nc.tensor.transpose(
                    tr[:, 0 : DOUT + 1],
                    ctxT_sb[:, ds(h * HQ + qt * P, P)],
                    ident[0 : DOUT + 1, 0 : DOUT + 1],
                )
                rc = opool.tile([P, 1], F32, tag="rc")
                nc.vector.reciprocal(rc, tr[:, DOUT : DOUT + 1])
                g, gi = qt // 4, qt % 4
                if gi == 0:
                    ostage[(h, g)] = opool.tile(
                        [P, 4, DOUT], F32, tag="ostage", name=f"ostage_{h}_{g}"
                    )
                nc.vector.tensor_scalar_mul(
                    ostage[(h, g)][:, gi, :], tr[:, 0:DOUT], rc
                )
                if gi == 3:
                    nc.sync.dma_start(
                        out=out[ds(h * HQ + g * 4 * P, 4 * P), :].rearrange(
                            "(t p) m -> p t m", p=P
                        ),
                        in_=ostage.pop((h, g)),
                    )

            # ---- main loop: q-half-major, 16 k-tiles inside
            ep_queue = []  # deferred epilogue work, interleaved into pass 1
            for h in range(H):
                ctx_ps = psC.tile([DOUT + 1, HQ], F32)
                for t in range(KT):
                    sc = psS.tile([P, HQ], F32, tag="big")
                    for n in range(HQ // NS):
                        qcols = ds(h * HQ + n * NS, NS)
                        nc.tensor.matmul(
                            sc[:, ds(n * NS, NS)],
                            lhsT=idr_sb,
                            rhs=mb_sb[:, t, :, qcols],
                            start=True,
                            stop=False,
                            perf_mode=DR,
                        )
                        nc.tensor.matmul(
                            sc[:, ds(n * NS, NS)],
                            lhsT=k_dr[:, :, ds(t * P, P)],
                            rhs=q_dr[:, :, qcols],
                            start=False,
                            stop=True,
                            perf_mode=DR,
                        )
                    ex = epool.tile([P, HQ], BF16, tag="exp")
                    nc.scalar.activation(
                        out=ex,
                        in_=sc,
                        func=mybir.ActivationFunctionType.Exp,
                        scale=inv_sqrt_s,
                    )
                    for n in range(HQ // NS):
                        nc.tensor.matmul(
                            ctx_ps[:, ds(n * NS, NS)],
                            lhsT=v65_sb[:, t, :],
                            rhs=ex[:, ds(n * NS, NS)],
                            start=(t == 0),
                            stop=(t == KT - 1),
                        )
                    if h == 0 and t < KT - 1:
                        vproj(t + 1)
                    # spread pass-0 epilogue work into pass 1
                    if ep_queue and t >= 1:
                        epilogue_tile(*ep_queue.pop(0))

                nc.vector.tensor_copy(ctxT_sb[:, ds(h * HQ, HQ)], ctx_ps)
                ep_queue.extend((h, qt) for qt in range(HQ // P))

            while ep_queue:
                epilogue_tile(*ep_queue.pop(0))

    nc.finalize()
    return nc


def _get_nc():
    global _NC_CACHE
    if _NC_CACHE is None:
        _NC_CACHE = build_nc()
    return _NC_CACHE


def kernel(**inputs):
    x = np.asarray(inputs["input_tensor"], dtype=np.float32)  # [B, S, DIN]
    mask = np.asarray(inputs["attention_mask"])  # [B, S, S] bool
    Wq = np.asarray(inputs["Wq"], dtype=np.float32)
    Wk = np.asarray(inputs["Wk"], dtype=np.float32)
    Wv = np.asarray(inputs["Wv"], dtype=np.float32)
    bq = np.asarray(inputs["bq"], dtype=np.float32)
    bk = np.asarray(inputs["bk"], dtype=np.float32)
    bv = np.asarray(inputs["bv"], dtype=np.float32)

    # Wqk in DR layout over din pairs: [j, p, g, m], din = 384g + 128j + p
    Wqk = np.concatenate([Wq, Wk], axis=1)  # [768, 128]
    wqk_h = np.ascontiguousarray(
        Wqk.reshape(2, NJ, P, P).transpose(1, 2, 0, 3)
    ).astype(ml_dtypes.float8_e4m3)
    wv_h = np.ascontiguousarray(Wv.reshape(NCH, P, DOUT)).astype(ml_dtypes.bfloat16)
    bqk_h = np.ascontiguousarray(np.concatenate([bq, bk]).reshape(P, 1))

    # DR identity for the additive mask matmul: idr[p, g, 64g+p] = 1
    idr_h = np.zeros((64, 2, P), dtype=np.float32)
    pp = np.arange(64)
    idr_h[pp, 0, pp] = 1.0
    idr_h[pp, 1, 64 + pp] = 1.0
    idr_h = idr_h.astype(ml_dtypes.float8_e5m2)

    in_maps = []
    for b in range(B):
        xT = np.ascontiguousarray(x[b].T)  # [DIN, S] fp32
        # x in DR fp8, block-major: [j, blk, p, g, s'], din = 384g+128j+p
        xdr_h = np.ascontiguousarray(
            xT.reshape(2, NJ, P, NB, NS).transpose(1, 3, 2, 0, 4)
        ).astype(ml_dtypes.float8_e4m3)
        xbf_h = np.ascontiguousarray(xT.reshape(NCH, P, S)).astype(ml_dtypes.bfloat16)
        # mask bias in DR fp8e5: [t, p, g, q], key = 128t + 64g + p
        maskT = mask[b].T  # [k, q]
        mb_h = np.ascontiguousarray(
            np.where(maskT, np.float32(-2048.0), np.float32(0.0))
            .reshape(KT, 2, 64, S)
            .transpose(0, 2, 1, 3)
        ).astype(ml_dtypes.float8_e5m2)
        in_maps.append(
            {
                "xdr": xdr_h,
                "xbf": xbf_h,
                "mb": mb_h,
                "wqk": wqk_h,
                "wv": wv_h,
                "idr": idr_h,
                "bqk": bqk_h,
            }
        )

    nc = _get_nc()
    res = run_bass_kernel_spmd(nc, in_maps, core_ids=list(range(B)))
    out = np.stack([np.asarray(res.results[b]["out"], np.float32) for b in range(B)])
    out = out + bv[None, None, :]
    return out.astype(np.float32)
